# revision 9
# baseline (speedup 1.0000x reference)
"""AffinityLoss on 8 TRN2 NeuronCores (Bass/Tile).

Math: for 3x3 unfold windows, aff = per-pixel 9x9 Gram over C=19 channels and
lb = label-equality; loss = mean BCE-with-logits. Reformulated as

    loss_sum = sum over 13 canonical relative shifts s=(dr,dc), positions (y,x):
        mult_s * wy_s(y) * wx_s(x) * [softplus(Corr_s(y,x)) - E_s(y,x)*Corr_s(y,x)]

where Corr_s(y,x) = sum_c A[c,y,x]*A[c,y+dr,x+dc], E_s = (labels equal across
the shift), wy/wx are small integer edge profiles, mult_s = 2 for s != (0,0).

Sharding: data-parallel over (image, row-half): core k owns image k//2, rows
[192*(k%2), 192*(k%2)+192). Per core: pass0 = 128 rows full width; pass1 =
64 rows split into 2 x-halves stacked on partitions. Per shift: bf16 product
tile [128, 19, W] -> tree-add over c -> corr; softplus = Relu + Ln(1+Exp(-Abs))
on the Scalar engine with accum_out row-sums; E*Corr row-sum via DVE mult +
ACT Identity accum. Border columns (wx deviates from its interior value only
within 2 of the edges) are exported exactly and corrected on the host in f64.
"""

import os
import sys

import numpy as np

for _p in ("/root/.axon_site", "/root/.axon_site/_ro/trn_rl_repo",
           "/root/.axon_site/_ro/pypackages"):
    if os.path.isdir(_p) and _p not in sys.path:
        sys.path.append(_p)

import ml_dtypes  # noqa: E402

N, C, H, W = 4, 19, 384, 384
K = 3
HP = WP = H - K + 1  # 382
N_CORES = 8
ROWS_PER_CORE = 192
SHIFTS = [(0, 0), (0, 1), (0, 2)] + [(dr, dc) for dr in (1, 2) for dc in (-2, -1, 0, 1, 2)]
NS = len(SHIFTS)  # 13
PASS_GEOM = [
    dict(row0=0, DW=W, TW=W + 8, groups=1),      # tiles [128, C, 392], data cols 2..385
    dict(row0=128, DW=194, TW=200, groups=2),    # 64 rows x 2 x-groups
]
GROUP_X0 = [0, 190]    # pass1 group g covers x in [190g, 190g+194)
NKIND = 3              # accs kinds {SPrelu, SPln, ECf}
COLS = 2 * NS * NKIND  # 78
BCPS = 12              # border cols per (pass, shift): rl 4, ln 4, ec 4
BCOLS = 2 * NS * BCPS  # 312
LGROWS = 196


def _wx_profile(dc, x):
    w = np.zeros_like(x, dtype=np.float64)
    for ca in range(K):
        if 0 <= ca + dc < K:
            w += ((x - ca >= 0) & (x - ca < WP))
    return w


def _wy_profile(dr, y):
    w = np.zeros_like(y, dtype=np.float64)
    for ra in range(K):
        if 0 <= ra + dr < K:
            w += ((y - ra >= 0) & (y - ra < HP))
    return w


def _border_weights():
    """bw[p, (pass*NS+s)*4 + bi]: (wx_eff - wxc) at window cols {0,1,DW-2,DW-1}."""
    bw = np.zeros((128, 2 * NS * 4), np.float64)
    for pi, geom in enumerate(PASS_GEOM):
        DW = geom["DW"]
        for si, (dr, dc) in enumerate(SHIFTS):
            wxc = sum(1 for ca in range(K) if 0 <= ca + dc < K)
            for p in range(128):
                if pi == 0:
                    gx0, own_lo, own_hi = 0, 0, W
                else:
                    g = p // 64
                    gx0 = GROUP_X0[g]
                    own_lo, own_hi = (0, 192) if g == 0 else (192, W)
                for bi, j in enumerate((0, 1, DW - 2, DW - 1)):
                    x = gx0 + j
                    if own_lo <= x < own_hi and 0 <= x + dc < W and x < W:
                        wx = _wx_profile(dc, np.array([x]))[0]
                    else:
                        wx = 0.0
                    bw[p, (pi * NS + si) * 4 + bi] = wx - wxc
    return bw


_BW = None


def _host_inputs(logits, labels):
    in_maps = []
    for k in range(N_CORES):
        img, half = k // 2, k % 2
        g0 = half * ROWS_PER_CORE
        hi = min(H, g0 + LGROWS)
        lg = np.zeros((C, LGROWS, W), np.float32)
        lg[:, : hi - g0] = logits[img, :, g0:hi]
        lb = np.full((LGROWS, W), -1.0, np.float32)
        lb[: hi - g0] = labels[img, g0:hi].astype(np.float32)
        in_maps.append({"lg": lg.astype(ml_dtypes.bfloat16), "lb": lb})
    return in_maps


def _combine(accs_list, bcols_list):
    global _BW
    if _BW is None:
        _BW = _border_weights()
    total = 0.0
    for k in range(N_CORES):
        acc = accs_list[k].astype(np.float64)
        bc = bcols_list[k].astype(np.float64)
        g0 = (k % 2) * ROWS_PER_CORE
        for pi in range(2):
            p = np.arange(128)
            gy = g0 + p if pi == 0 else g0 + 128 + (p % 64)
            for si, (dr, dc) in enumerate(SHIFTS):
                mult = 1.0 if (dr, dc) == (0, 0) else 2.0
                wxc = float(sum(1 for ca in range(K) if 0 <= ca + dc < K))
                wy = _wy_profile(dr, gy)
                base = (pi * NS + si) * NKIND
                bb = (pi * NS + si) * BCPS
                wb = _BW[:, (pi * NS + si) * 4:(pi * NS + si) * 4 + 4]
                spf = acc[:, base + 0] + acc[:, base + 1]
                ecf = acc[:, base + 2]
                spb = ((bc[:, bb + 0:bb + 4] + bc[:, bb + 4:bb + 8]) * wb).sum(1)
                ecb = (bc[:, bb + 8:bb + 12] * wb).sum(1)
                total += mult * np.sum(wy * (wxc * (spf - ecf) + (spb - ecb)))
    return total / (N * 81 * HP * WP)


_NC = None


def _build():
    global _NC
    if _NC is not None:
        return _NC
    from concourse import bacc, mybir
    import concourse.tile as tile

    f32 = mybir.dt.float32
    bf16 = mybir.dt.bfloat16
    Alu = mybir.AluOpType
    AF = mybir.ActivationFunctionType

    nc = bacc.Bacc("TRN2", target_bir_lowering=False, debug=False, num_devices=N_CORES)
    lg = nc.dram_tensor("lg", (C, LGROWS, W), bf16, kind="ExternalInput")
    lb = nc.dram_tensor("lb", (LGROWS, W), f32, kind="ExternalInput")
    accs = nc.dram_tensor("accs", (128, COLS), f32, kind="ExternalOutput")
    bcols = nc.dram_tensor("bcols", (128, BCOLS), f32, kind="ExternalOutput")

    with tile.TileContext(nc) as tc:
        with tc.tile_pool(name="persist", bufs=1) as pool, \
             tc.tile_pool(name="work", bufs=2) as wpool:
            accs_t = pool.tile([128, COLS], f32, name="accs_t")
            bcols_t = pool.tile([128, BCOLS], f32, name="bcols_t")

            for pi, geom in enumerate(PASS_GEOM):
                row0, DW, TW, groups = geom["row0"], geom["DW"], geom["TW"], geom["groups"]
                T, Tod, L = {}, {}, {}
                for dr in range(K):
                    t = pool.tile([128, C, TW], bf16, tag=f"T{dr}", name=f"T{dr}_{pi}")
                    to = pool.tile([128, C, TW], bf16, tag=f"O{dr}", name=f"O{dr}_{pi}")
                    lt = pool.tile([128, TW], f32, tag=f"L{dr}", name=f"L{dr}_{pi}")
                    nc.gpsimd.memset(t[:, :, 0:2], 0)
                    nc.gpsimd.memset(t[:, :, 2 + DW:TW], 0)
                    nc.gpsimd.memset(to[:, :, 0:1], 0)
                    nc.gpsimd.memset(to[:, :, 1 + DW:TW], 0)
                    nc.gpsimd.memset(lt[:, 0:2], 0)
                    nc.gpsimd.memset(lt[:, 2 + DW:TW], 0)
                    if groups == 1:
                        src = lg[:, row0 + dr:row0 + dr + 128, :].rearrange("c y x -> y c x")
                        nc.sync.dma_start(t[:, :, 2:2 + DW], src)
                        nc.sync.dma_start(to[:, :, 1:1 + DW], src)
                        nc.sync.dma_start(lt[:, 2:2 + DW], lb[row0 + dr:row0 + dr + 128, :])
                    else:
                        for g in range(groups):
                            x0 = GROUP_X0[g]
                            src = lg[:, row0 + dr:row0 + dr + 64, x0:x0 + DW].rearrange(
                                "c y x -> y c x")
                            nc.sync.dma_start(t[64 * g:64 * g + 64, :, 2:2 + DW], src)
                            nc.sync.dma_start(to[64 * g:64 * g + 64, :, 1:1 + DW], src)
                            nc.sync.dma_start(
                                lt[64 * g:64 * g + 64, 2:2 + DW],
                                lb[row0 + dr:row0 + dr + 64, x0:x0 + DW])
                    T[dr], Tod[dr], L[dr] = t, to, lt

                for si, (dr, dc) in enumerate(SHIFTS):
                    base = (pi * NS + si) * NKIND
                    bb = (pi * NS + si) * BCPS
                    odd = (dc % 2) != 0
                    t1 = Tod[dr] if odd else T[dr]
                    o1 = (1 + dc) if odd else (2 + dc)

                    pb = wpool.tile([128, C, DW], bf16, tag="pb", name=f"pb_{pi}_{si}")
                    corr = wpool.tile([128, DW], f32, tag="corr", name=f"corr_{pi}_{si}")
                    e = wpool.tile([128, DW], f32, tag="e", name=f"e_{pi}_{si}")
                    av = wpool.tile([128, DW], f32, tag="av", name=f"av_{pi}_{si}")
                    ex = wpool.tile([128, DW], f32, tag="ex", name=f"ex_{pi}_{si}")
                    ln1 = wpool.tile([128, DW], f32, tag="ln1", name=f"ln1_{pi}_{si}")
                    rl = wpool.tile([128, DW], f32, tag="rl", name=f"rl_{pi}_{si}")
                    ec = wpool.tile([128, DW], f32, tag="ec", name=f"ec_{pi}_{si}")
                    ecs = wpool.tile([128, DW], f32, tag="ecs", name=f"ecs_{pi}_{si}")

                    nc.vector.tensor_tensor(
                        pb[:, :, :], T[0][:, :, 2:2 + DW], t1[:, :, o1:o1 + DW], Alu.mult)
                    nc.vector.tensor_tensor(
                        pb[:, 0:9, :], pb[:, 0:9, :], pb[:, 9:18, :], Alu.add)
                    nc.vector.tensor_tensor(
                        pb[:, 0:4, :], pb[:, 0:4, :], pb[:, 4:8, :], Alu.add)
                    nc.vector.tensor_tensor(
                        pb[:, 0:2, :], pb[:, 0:2, :], pb[:, 2:4, :], Alu.add)
                    nc.vector.tensor_tensor(
                        pb[:, 0:1, :], pb[:, 0:1, :], pb[:, 1:2, :], Alu.add)
                    nc.vector.tensor_tensor(
                        pb[:, 0:1, :], pb[:, 0:1, :], pb[:, 8:9, :], Alu.add)
                    nc.vector.tensor_tensor(
                        corr[:, 0:DW], pb[:, 0:1, :], pb[:, 18:19, :], Alu.add)

                    nc.vector.tensor_tensor(
                        e[:, 0:DW], L[0][:, 2:2 + DW], L[dr][:, 2 + dc:2 + dc + DW],
                        Alu.is_equal)
                    nc.vector.tensor_tensor(
                        ec[:, 0:DW], corr[:, 0:DW], e[:, 0:DW], Alu.mult)

                    # softplus(corr) = relu(corr) + ln(1 + exp(-|corr|)) on ScalarE
                    nc.scalar.activation(
                        rl[:, 0:DW], corr[:, 0:DW], AF.Relu,
                        accum_out=accs_t[:, base + 0:base + 1])
                    nc.scalar.activation(av[:, 0:DW], corr[:, 0:DW], AF.Abs)
                    nc.scalar.activation(ex[:, 0:DW], av[:, 0:DW], AF.Exp, scale=-1.0)
                    nc.scalar.activation(
                        ln1[:, 0:DW], ex[:, 0:DW], AF.Ln, bias=1.0,
                        accum_out=accs_t[:, base + 1:base + 2])
                    nc.scalar.activation(
                        ecs[:, 0:DW], ec[:, 0:DW], AF.Identity,
                        accum_out=accs_t[:, base + 2:base + 3])
                    # export exact border cols {0,1,DW-2,DW-1} of rl/ln1/ec
                    for ti, tt in enumerate((rl, ln1, ec)):
                        nc.scalar.activation(
                            bcols_t[:, bb + 4 * ti:bb + 4 * ti + 2], tt[:, 0:2], AF.Copy)
                        nc.scalar.activation(
                            bcols_t[:, bb + 4 * ti + 2:bb + 4 * ti + 4],
                            tt[:, DW - 2:DW], AF.Copy)

            nc.sync.dma_start(accs[:, :], accs_t[:])
            nc.sync.dma_start(bcols[:, :], bcols_t[:])

    nc.finalize()
    _NC = nc
    return nc


def kernel(logits, labels):
    nc = _build()
    in_maps = _host_inputs(np.asarray(logits, np.float32), np.asarray(labels))
    from concourse.bass_utils import run_bass_kernel_spmd
    res = run_bass_kernel_spmd(nc, in_maps, core_ids=list(range(N_CORES)))
    accs_list = [res.results[k]["accs"] for k in range(N_CORES)]
    bcols_list = [res.results[k]["bcols"] for k in range(N_CORES)]
    return np.array(_combine(accs_list, bcols_list), np.float32)


# revision 11
# speedup vs baseline: 1.0938x; 1.0938x over previous
"""AffinityLoss on 8 TRN2 NeuronCores (Bass/Tile).

Math: 3x3-unfold affinity loss = mean BCE-with-logits between per-pixel 9x9
channel Gram matrices and label-equality maps. Reformulated over 13 canonical
relative shifts s=(dr,dc) with integer edge-weight profiles wy/wx:

    loss_sum = sum_s mult_s * sum_{y,x} wy_s(y) wx_s(x) * ln(1 + exp(w_s(y,x)))
    w_s = (1 - 2*E_s) * Corr_s,   Corr_s(y,x) = sum_c A[c,y,x] A[c,y+dr,x+dc],
    E_s = [labels equal across the shift]

(BCE-with-logits == softplus((1-2E)*x); max |Corr| ~ 66 so exp is safe in f32.)

Sharding: data-parallel, core k owns image k//2, rows [192*(k%2), +192).
Per core: pass0 = 128 rows full width; pass1 = 64 rows x 2 x-half groups
stacked on partitions. Per shift: bf16 product tile [128,19,W] (DVE mult, or
ScalarE Square for s=(0,0)) -> bf16 tree-add over c -> corr; sign w from
labels (is_equal + affine + mult); ScalarE Exp then Ln(bias=1, accum_out) give
the per-row loss sums. wx border deviations live within 2 cols of the edges;
those columns are exported exactly (GPSIMD copies) and corrected on host in
f64 together with the wy weighting and the cross-core reduction.
"""

import os
import sys

import numpy as np

for _p in ("/root/.axon_site", "/root/.axon_site/_ro/trn_rl_repo",
           "/root/.axon_site/_ro/pypackages"):
    if os.path.isdir(_p) and _p not in sys.path:
        sys.path.append(_p)

import ml_dtypes  # noqa: E402

N, C, H, W = 4, 19, 384, 384
K = 3
HP = WP = H - K + 1  # 382
N_CORES = 8
ROWS_PER_CORE = 192
SHIFTS = [(0, 0), (0, 1), (0, 2)] + [(dr, dc) for dr in (1, 2) for dc in (-2, -1, 0, 1, 2)]
NS = len(SHIFTS)  # 13
PASS_GEOM = [
    dict(row0=0, DW=W, TW=W + 8, groups=1),      # tiles [128, C, 392], data cols 2..385
    dict(row0=128, DW=194, TW=200, groups=2),    # 64 rows x 2 x-groups
]
GROUP_X0 = [0, 190]   # pass1 group g covers x in [190g, 190g+194)
COLS = 2 * NS        # one ln-accum column per (pass, shift)
BCOLS = 2 * NS * 4   # 4 exported border cols per (pass, shift)
LGROWS = 196


def _wx_profile(dc, x):
    w = np.zeros_like(x, dtype=np.float64)
    for ca in range(K):
        if 0 <= ca + dc < K:
            w += ((x - ca >= 0) & (x - ca < WP))
    return w


def _wy_profile(dr, y):
    w = np.zeros_like(y, dtype=np.float64)
    for ra in range(K):
        if 0 <= ra + dr < K:
            w += ((y - ra >= 0) & (y - ra < HP))
    return w


def _border_weights():
    """bw[p, (pass*NS+s)*4 + bi]: (wx_eff - wxc) at window cols {0,1,DW-2,DW-1}."""
    bw = np.zeros((128, 2 * NS * 4), np.float64)
    for pi, geom in enumerate(PASS_GEOM):
        DW = geom["DW"]
        for si, (dr, dc) in enumerate(SHIFTS):
            wxc = sum(1 for ca in range(K) if 0 <= ca + dc < K)
            for p in range(128):
                if pi == 0:
                    gx0, own_lo, own_hi = 0, 0, W
                else:
                    g = p // 64
                    gx0 = GROUP_X0[g]
                    own_lo, own_hi = (0, 192) if g == 0 else (192, W)
                for bi, j in enumerate((0, 1, DW - 2, DW - 1)):
                    x = gx0 + j
                    if own_lo <= x < own_hi and 0 <= x + dc < W and x < W:
                        wx = _wx_profile(dc, np.array([x]))[0]
                    else:
                        wx = 0.0
                    bw[p, (pi * NS + si) * 4 + bi] = wx - wxc
    return bw


_BW = None


def _host_inputs(logits, labels):
    in_maps = []
    for k in range(N_CORES):
        img, half = k // 2, k % 2
        g0 = half * ROWS_PER_CORE
        hi = min(H, g0 + LGROWS)
        lg = np.zeros((C, LGROWS, W), np.float32)
        lg[:, : hi - g0] = logits[img, :, g0:hi]
        lb = np.full((LGROWS, W), -1.0, np.float32)
        lb[: hi - g0] = labels[img, g0:hi].astype(np.float32)
        in_maps.append({
            "lg": lg.astype(ml_dtypes.bfloat16),
            "lb": lb.astype(ml_dtypes.bfloat16),
        })
    return in_maps


def _combine(accs_list, bcols_list):
    global _BW
    if _BW is None:
        _BW = _border_weights()
    total = 0.0
    for k in range(N_CORES):
        acc = accs_list[k].astype(np.float64)
        bc = bcols_list[k].astype(np.float64)
        g0 = (k % 2) * ROWS_PER_CORE
        for pi in range(2):
            p = np.arange(128)
            gy = g0 + p if pi == 0 else g0 + 128 + (p % 64)
            for si, (dr, dc) in enumerate(SHIFTS):
                mult = 1.0 if (dr, dc) == (0, 0) else 2.0
                wxc = float(sum(1 for ca in range(K) if 0 <= ca + dc < K))
                wy = _wy_profile(dr, gy)
                idx = pi * NS + si
                wb = _BW[:, idx * 4: idx * 4 + 4]
                full = acc[:, idx]
                border = (bc[:, idx * 4: idx * 4 + 4] * wb).sum(1)
                total += mult * np.sum(wy * (wxc * full + border))
    return total / (N * 81 * HP * WP)


_NC = None


def _build():
    global _NC
    if _NC is not None:
        return _NC
    from concourse import bacc, mybir
    import concourse.tile as tile

    f32 = mybir.dt.float32
    bf16 = mybir.dt.bfloat16
    Alu = mybir.AluOpType
    AF = mybir.ActivationFunctionType

    nc = bacc.Bacc("TRN2", target_bir_lowering=False, debug=False, num_devices=N_CORES)
    lg = nc.dram_tensor("lg", (C, LGROWS, W), bf16, kind="ExternalInput")
    lb = nc.dram_tensor("lb", (LGROWS, W), bf16, kind="ExternalInput")
    accs = nc.dram_tensor("accs", (128, COLS), f32, kind="ExternalOutput")
    bcols = nc.dram_tensor("bcols", (128, BCOLS), f32, kind="ExternalOutput")

    with tile.TileContext(nc) as tc:
        with tc.tile_pool(name="persist", bufs=1) as pool, \
             tc.tile_pool(name="work", bufs=2) as wpool:
            accs_t = pool.tile([128, COLS], f32, name="accs_t")
            bcols_t = pool.tile([128, BCOLS], f32, name="bcols_t")

            for pi, geom in enumerate(PASS_GEOM):
                row0, DW, TW, groups = geom["row0"], geom["DW"], geom["TW"], geom["groups"]
                T, Tod, L = {}, {}, {}
                for dr in range(K):
                    t = pool.tile([128, C, TW], bf16, tag=f"T{dr}", name=f"T{dr}_{pi}")
                    to = pool.tile([128, C, TW], bf16, tag=f"O{dr}", name=f"O{dr}_{pi}")
                    lt = pool.tile([128, TW], bf16, tag=f"L{dr}", name=f"L{dr}_{pi}")
                    nc.gpsimd.memset(t[:, :, 0:2], 0)
                    nc.gpsimd.memset(t[:, :, 2 + DW:TW], 0)
                    nc.gpsimd.memset(to[:, :, 0:1], 0)
                    nc.gpsimd.memset(to[:, :, 1 + DW:TW], 0)
                    nc.gpsimd.memset(lt[:, 0:2], 0)
                    nc.gpsimd.memset(lt[:, 2 + DW:TW], 0)
                    if groups == 1:
                        src = lg[:, row0 + dr:row0 + dr + 128, :].rearrange("c y x -> y c x")
                        nc.sync.dma_start(t[:, :, 2:2 + DW], src)
                        nc.sync.dma_start(to[:, :, 1:1 + DW], src)
                        nc.sync.dma_start(lt[:, 2:2 + DW], lb[row0 + dr:row0 + dr + 128, :])
                    else:
                        for g in range(groups):
                            x0 = GROUP_X0[g]
                            src = lg[:, row0 + dr:row0 + dr + 64, x0:x0 + DW].rearrange(
                                "c y x -> y c x")
                            nc.sync.dma_start(t[64 * g:64 * g + 64, :, 2:2 + DW], src)
                            nc.sync.dma_start(to[64 * g:64 * g + 64, :, 1:1 + DW], src)
                            nc.sync.dma_start(
                                lt[64 * g:64 * g + 64, 2:2 + DW],
                                lb[row0 + dr:row0 + dr + 64, x0:x0 + DW])
                    T[dr], Tod[dr], L[dr] = t, to, lt

                for si, (dr, dc) in enumerate(SHIFTS):
                    idx = pi * NS + si
                    odd = (dc % 2) != 0
                    t1 = Tod[dr] if odd else T[dr]
                    o1 = (1 + dc) if odd else (2 + dc)

                    pb = wpool.tile([128, C, DW], bf16, tag="pb", name=f"pb_{pi}_{si}")
                    corr = wpool.tile([128, DW], bf16, tag="corr", name=f"corr_{pi}_{si}")
                    u = wpool.tile([128, DW], f32, tag="u", name=f"u_{pi}_{si}")
                    l1 = wpool.tile([128, DW], f32, tag="l1", name=f"l1_{pi}_{si}")

                    if si == 0:
                        # Corr_00 = sum_c A_c^2: products on ScalarE (Square)
                        nc.scalar.activation(
                            pb[:, :, :], T[0][:, :, 2:2 + DW], AF.Square)
                    else:
                        nc.vector.tensor_tensor(
                            pb[:, :, :], T[0][:, :, 2:2 + DW], t1[:, :, o1:o1 + DW],
                            Alu.mult)
                    nc.vector.tensor_tensor(
                        pb[:, 0:9, :], pb[:, 0:9, :], pb[:, 9:18, :], Alu.add)
                    nc.vector.tensor_tensor(
                        pb[:, 0:4, :], pb[:, 0:4, :], pb[:, 4:8, :], Alu.add)
                    nc.vector.tensor_tensor(
                        pb[:, 0:2, :], pb[:, 0:2, :], pb[:, 2:4, :], Alu.add)
                    nc.vector.tensor_tensor(
                        pb[:, 0:1, :], pb[:, 0:1, :], pb[:, 1:2, :], Alu.add)
                    nc.vector.tensor_tensor(
                        pb[:, 0:1, :], pb[:, 0:1, :], pb[:, 8:9, :], Alu.add)
                    nc.vector.tensor_tensor(
                        corr[:, 0:DW], pb[:, 0:1, :], pb[:, 18:19, :], Alu.add)

                    if si == 0:
                        # E == 1 everywhere: w = -corr, fold into Exp's scale
                        nc.scalar.activation(u[:, 0:DW], corr[:, 0:DW], AF.Exp,
                                             scale=-1.0)
                    else:
                        e = wpool.tile([128, DW], bf16, tag="e", name=f"e_{pi}_{si}")
                        sh = wpool.tile([128, DW], bf16, tag="sh", name=f"sh_{pi}_{si}")
                        wt = wpool.tile([128, DW], bf16, tag="wt", name=f"wt_{pi}_{si}")
                        nc.vector.tensor_tensor(
                            e[:, 0:DW], L[0][:, 2:2 + DW], L[dr][:, 2 + dc:2 + dc + DW],
                            Alu.is_equal)
                        nc.vector.tensor_scalar(
                            sh[:, 0:DW], e[:, 0:DW], -2.0, 1.0, Alu.mult, Alu.add)
                        nc.vector.tensor_tensor(
                            wt[:, 0:DW], sh[:, 0:DW], corr[:, 0:DW], Alu.mult)
                        nc.scalar.activation(u[:, 0:DW], wt[:, 0:DW], AF.Exp)
                    nc.scalar.activation(
                        l1[:, 0:DW], u[:, 0:DW], AF.Ln, bias=1.0,
                        accum_out=accs_t[:, idx:idx + 1])
                    nc.gpsimd.tensor_copy(bcols_t[:, idx * 4:idx * 4 + 2], l1[:, 0:2])
                    nc.gpsimd.tensor_copy(bcols_t[:, idx * 4 + 2:idx * 4 + 4],
                                          l1[:, DW - 2:DW])

            nc.sync.dma_start(accs[:, :], accs_t[:])
            nc.sync.dma_start(bcols[:, :], bcols_t[:])

    nc.finalize()
    _NC = nc
    return nc


def kernel(logits, labels):
    nc = _build()
    in_maps = _host_inputs(np.asarray(logits, np.float32), np.asarray(labels))
    from concourse.bass_utils import run_bass_kernel_spmd
    res = run_bass_kernel_spmd(nc, in_maps, core_ids=list(range(N_CORES)))
    accs_list = [res.results[k]["accs"] for k in range(N_CORES)]
    bcols_list = [res.results[k]["bcols"] for k in range(N_CORES)]
    return np.array(_combine(accs_list, bcols_list), np.float32)


# revision 14
# speedup vs baseline: 1.1765x; 1.0756x over previous
"""AffinityLoss on 8 TRN2 NeuronCores (Bass/Tile).

Math: 3x3-unfold affinity loss = mean BCE-with-logits between per-pixel 9x9
channel Gram matrices and label-equality maps. Reformulated over 13 canonical
relative shifts s=(dr,dc) with integer edge-weight profiles wy/wx:

    loss_sum = sum_s mult_s * sum_{y,x} wy_s(y) wx_s(x) * ln(1 + exp(w_s(y,x)))
    w_s = (1 - 2*E_s) * Corr_s,   Corr_s(y,x) = sum_c A[c,y,x] A[c,y+dr,x+dc],
    E_s = [labels equal across the shift]

(BCE-with-logits == softplus((1-2E)*x); max |Corr| ~ 66 so exp is safe in f32.)

Sharding: data-parallel, core k owns image k//2, rows [192*(k%2), +192).
Per core: pass0 = 128 rows full width; pass1 = 64 rows x 2 x-half groups
stacked on partitions. Per shift: bf16 product tile [128,19,W] (DVE mult, or
ScalarE Square for s=(0,0)) -> bf16 tree-add over c -> corr; sign w from
labels (is_equal + affine + mult); ScalarE Exp then Ln(bias=1, accum_out) give
the per-row loss sums. wx border deviations live within 2 cols of the edges;
those columns are exported exactly (GPSIMD copies) and corrected on host in
f64 together with the wy weighting and the cross-core reduction.
"""

import os
import sys

import numpy as np

for _p in ("/root/.axon_site", "/root/.axon_site/_ro/trn_rl_repo",
           "/root/.axon_site/_ro/pypackages"):
    if os.path.isdir(_p) and _p not in sys.path:
        sys.path.append(_p)

import ml_dtypes  # noqa: E402

N, C, H, W = 4, 19, 384, 384
K = 3
HP = WP = H - K + 1  # 382
N_CORES = 8
ROWS_PER_CORE = 192
SHIFTS = [(0, 0), (0, 1), (0, 2)] + [(dr, dc) for dr in (1, 2) for dc in (-2, -1, 0, 1, 2)]
NS = len(SHIFTS)  # 13
PASS_GEOM = [
    dict(row0=0, DW=W, TW=W + 8, groups=1),      # tiles [128, C, 392], data cols 2..385
    dict(row0=128, DW=194, TW=200, groups=2),    # 64 rows x 2 x-groups
]
GROUP_X0 = [0, 190]   # pass1 group g covers x in [190g, 190g+194)
COLS = 2 * NS        # one ln-accum column per (pass, shift)
BCOLS = 2 * NS * 4   # 4 exported border cols per (pass, shift)
LGROWS = 196


def _wx_profile(dc, x):
    w = np.zeros_like(x, dtype=np.float64)
    for ca in range(K):
        if 0 <= ca + dc < K:
            w += ((x - ca >= 0) & (x - ca < WP))
    return w


def _wy_profile(dr, y):
    w = np.zeros_like(y, dtype=np.float64)
    for ra in range(K):
        if 0 <= ra + dr < K:
            w += ((y - ra >= 0) & (y - ra < HP))
    return w


def _border_weights():
    """bw[p, (pass*NS+s)*4 + bi]: (wx_eff - wxc) at window cols {0,1,DW-2,DW-1}."""
    bw = np.zeros((128, 2 * NS * 4), np.float64)
    for pi, geom in enumerate(PASS_GEOM):
        DW = geom["DW"]
        for si, (dr, dc) in enumerate(SHIFTS):
            wxc = sum(1 for ca in range(K) if 0 <= ca + dc < K)
            for p in range(128):
                if pi == 0:
                    gx0, own_lo, own_hi = 0, 0, W
                else:
                    g = p // 64
                    gx0 = GROUP_X0[g]
                    own_lo, own_hi = (0, 192) if g == 0 else (192, W)
                for bi, j in enumerate((0, 1, DW - 2, DW - 1)):
                    x = gx0 + j
                    if own_lo <= x < own_hi and 0 <= x + dc < W and x < W:
                        wx = _wx_profile(dc, np.array([x]))[0]
                    else:
                        wx = 0.0
                    bw[p, (pi * NS + si) * 4 + bi] = wx - wxc
    return bw


_BW = None


def _host_inputs(logits, labels):
    in_maps = []
    for k in range(N_CORES):
        img, half = k // 2, k % 2
        g0 = half * ROWS_PER_CORE
        hi = min(H, g0 + LGROWS)
        lg = np.zeros((C, LGROWS, W), np.float32)
        lg[:, : hi - g0] = logits[img, :, g0:hi]
        lb = np.full((LGROWS, W), -1.0, np.float32)
        lb[: hi - g0] = labels[img, g0:hi].astype(np.float32)
        in_maps.append({
            "lg": lg.astype(ml_dtypes.bfloat16),
            "lb": lb.astype(ml_dtypes.bfloat16),
        })
    return in_maps


def _combine(accs_list, bcols_list):
    global _BW
    if _BW is None:
        _BW = _border_weights()
    total = 0.0
    for k in range(N_CORES):
        acc = accs_list[k].astype(np.float64)
        bc = bcols_list[k].astype(np.float64)
        g0 = (k % 2) * ROWS_PER_CORE
        for pi in range(2):
            p = np.arange(128)
            gy = g0 + p if pi == 0 else g0 + 128 + (p % 64)
            for si, (dr, dc) in enumerate(SHIFTS):
                mult = 1.0 if (dr, dc) == (0, 0) else 2.0
                wxc = float(sum(1 for ca in range(K) if 0 <= ca + dc < K))
                wy = _wy_profile(dr, gy)
                idx = pi * NS + si
                wb = _BW[:, idx * 4: idx * 4 + 4]
                full = acc[:, idx]
                border = (bc[:, idx * 4: idx * 4 + 4] * wb).sum(1)
                total += mult * np.sum(wy * (wxc * full + border))
    return total / (N * 81 * HP * WP)


_NC = None


def _build():
    global _NC
    if _NC is not None:
        return _NC
    from concourse import bacc, mybir
    import concourse.tile as tile

    f32 = mybir.dt.float32
    bf16 = mybir.dt.bfloat16
    Alu = mybir.AluOpType
    AF = mybir.ActivationFunctionType

    # All activations used here (Exp, Ln, Square, Copy) live together in the
    # "natural_log_exp_and_others" table set, but the table-load pass resolves
    # each function to the FIRST set containing it, which alternates sets and
    # reloads the ACT tables before nearly every activation (~80us of
    # ACT_TABLE_LOAD).  Filter the other sets' membership (indices untouched)
    # so everything resolves to the one shared set -> a single load.
    from concourse.hw_specs import get_activation_tables as _gat
    _keep = "natural_log_exp_and_others"
    _mine = {AF.Exp, AF.Ln, AF.Square, AF.Copy}

    def _gat_filtered(arch):
        t = _gat(arch)
        for name in t:
            if name != _keep:
                t[name] = t[name] - _mine
        return t

    bacc.get_activation_tables = _gat_filtered

    nc = bacc.Bacc("TRN2", target_bir_lowering=False, debug=False, num_devices=N_CORES)
    lg = nc.dram_tensor("lg", (C, LGROWS, W), bf16, kind="ExternalInput")
    lb = nc.dram_tensor("lb", (LGROWS, W), bf16, kind="ExternalInput")
    accs = nc.dram_tensor("accs", (128, COLS), f32, kind="ExternalOutput")
    bcols = nc.dram_tensor("bcols", (128, BCOLS), f32, kind="ExternalOutput")

    with tile.TileContext(nc) as tc:
        with tc.tile_pool(name="persist", bufs=1) as pool, \
             tc.tile_pool(name="work", bufs=2) as wpool:
            accs_t = pool.tile([128, COLS], f32, name="accs_t")
            bcols_t = pool.tile([128, BCOLS], f32, name="bcols_t")

            for pi, geom in enumerate(PASS_GEOM):
                row0, DW, TW, groups = geom["row0"], geom["DW"], geom["TW"], geom["groups"]
                T, Tod, L = {}, {}, {}
                for dr in range(K):
                    t = pool.tile([128, C, TW], bf16, tag=f"T{dr}_{pi}", name=f"T{dr}_{pi}")
                    to = pool.tile([128, C, TW], bf16, tag=f"O{dr}_{pi}", name=f"O{dr}_{pi}")
                    lt = pool.tile([128, TW], bf16, tag=f"L{dr}_{pi}", name=f"L{dr}_{pi}")
                    nc.gpsimd.memset(t[:, :, 0:2], 0)
                    nc.gpsimd.memset(t[:, :, 2 + DW:TW], 0)
                    nc.gpsimd.memset(to[:, :, 0:1], 0)
                    nc.gpsimd.memset(to[:, :, 1 + DW:TW], 0)
                    nc.gpsimd.memset(lt[:, 0:2], 0)
                    nc.gpsimd.memset(lt[:, 2 + DW:TW], 0)
                    if groups == 1:
                        src = lg[:, row0 + dr:row0 + dr + 128, :].rearrange("c y x -> y c x")
                        nc.sync.dma_start(t[:, :, 2:2 + DW], src)
                        nc.sync.dma_start(to[:, :, 1:1 + DW], src)
                        nc.sync.dma_start(lt[:, 2:2 + DW], lb[row0 + dr:row0 + dr + 128, :])
                    else:
                        for g in range(groups):
                            x0 = GROUP_X0[g]
                            src = lg[:, row0 + dr:row0 + dr + 64, x0:x0 + DW].rearrange(
                                "c y x -> y c x")
                            nc.sync.dma_start(t[64 * g:64 * g + 64, :, 2:2 + DW], src)
                            nc.sync.dma_start(to[64 * g:64 * g + 64, :, 1:1 + DW], src)
                            nc.sync.dma_start(
                                lt[64 * g:64 * g + 64, 2:2 + DW],
                                lb[row0 + dr:row0 + dr + 64, x0:x0 + DW])
                    T[dr], Tod[dr], L[dr] = t, to, lt

                for si, (dr, dc) in enumerate(SHIFTS):
                    idx = pi * NS + si
                    odd = (dc % 2) != 0
                    t1 = Tod[dr] if odd else T[dr]
                    o1 = (1 + dc) if odd else (2 + dc)

                    pb = wpool.tile([128, C, DW], bf16, tag="pb", name=f"pb_{pi}_{si}")
                    corr = wpool.tile([128, DW], bf16, tag="corr", name=f"corr_{pi}_{si}")
                    u = wpool.tile([128, DW], f32, tag="u", name=f"u_{pi}_{si}")
                    l1 = wpool.tile([128, DW], f32, tag="l1", name=f"l1_{pi}_{si}")

                    if si == 0:
                        # Corr_00 = sum_c A_c^2: products on ScalarE (Square)
                        nc.scalar.activation(
                            pb[:, :, :], T[0][:, :, 2:2 + DW], AF.Square)
                    else:
                        nc.vector.tensor_tensor(
                            pb[:, :, :], T[0][:, :, 2:2 + DW], t1[:, :, o1:o1 + DW],
                            Alu.mult)
                    nc.vector.tensor_tensor(
                        pb[:, 0:9, :], pb[:, 0:9, :], pb[:, 9:18, :], Alu.add)
                    nc.vector.tensor_tensor(
                        pb[:, 0:4, :], pb[:, 0:4, :], pb[:, 4:8, :], Alu.add)
                    nc.vector.tensor_tensor(
                        pb[:, 0:2, :], pb[:, 0:2, :], pb[:, 2:4, :], Alu.add)
                    nc.vector.tensor_tensor(
                        pb[:, 0:1, :], pb[:, 0:1, :], pb[:, 1:2, :], Alu.add)
                    nc.vector.tensor_tensor(
                        pb[:, 0:1, :], pb[:, 0:1, :], pb[:, 8:9, :], Alu.add)
                    nc.vector.tensor_tensor(
                        corr[:, 0:DW], pb[:, 0:1, :], pb[:, 18:19, :], Alu.add)

                    if si == 0:
                        # E == 1 everywhere: w = -corr, fold into Exp's scale
                        nc.scalar.activation(u[:, 0:DW], corr[:, 0:DW], AF.Exp,
                                             scale=-1.0)
                    else:
                        e = wpool.tile([128, DW], bf16, tag="e", name=f"e_{pi}_{si}")
                        sh = wpool.tile([128, DW], bf16, tag="sh", name=f"sh_{pi}_{si}")
                        wt = wpool.tile([128, DW], bf16, tag="wt", name=f"wt_{pi}_{si}")
                        nc.vector.tensor_tensor(
                            e[:, 0:DW], L[0][:, 2:2 + DW], L[dr][:, 2 + dc:2 + dc + DW],
                            Alu.is_equal)
                        nc.vector.tensor_scalar(
                            sh[:, 0:DW], e[:, 0:DW], -2.0, 1.0, Alu.mult, Alu.add)
                        nc.vector.tensor_tensor(
                            wt[:, 0:DW], sh[:, 0:DW], corr[:, 0:DW], Alu.mult)
                        nc.scalar.activation(u[:, 0:DW], wt[:, 0:DW], AF.Exp)
                    nc.scalar.activation(
                        l1[:, 0:DW], u[:, 0:DW], AF.Ln, bias=1.0,
                        accum_out=accs_t[:, idx:idx + 1])
                    nc.scalar.copy(bcols_t[:, idx * 4:idx * 4 + 2], l1[:, 0:2])
                    nc.scalar.copy(bcols_t[:, idx * 4 + 2:idx * 4 + 4],
                                   l1[:, DW - 2:DW])

            nc.sync.dma_start(accs[:, :], accs_t[:])
            nc.sync.dma_start(bcols[:, :], bcols_t[:])

    nc.finalize()
    _NC = nc
    return nc


def kernel(logits, labels):
    nc = _build()
    in_maps = _host_inputs(np.asarray(logits, np.float32), np.asarray(labels))
    from concourse.bass_utils import run_bass_kernel_spmd
    res = run_bass_kernel_spmd(nc, in_maps, core_ids=list(range(N_CORES)))
    accs_list = [res.results[k]["accs"] for k in range(N_CORES)]
    bcols_list = [res.results[k]["bcols"] for k in range(N_CORES)]
    return np.array(_combine(accs_list, bcols_list), np.float32)


# revision 15
# speedup vs baseline: 1.1974x; 1.0178x over previous
"""AffinityLoss on 8 TRN2 NeuronCores (Bass/Tile).

Math: 3x3-unfold affinity loss = mean BCE-with-logits between per-pixel 9x9
channel Gram matrices and label-equality maps. Reformulated over 13 canonical
relative shifts s=(dr,dc) with integer edge-weight profiles wy/wx:

    loss_sum = sum_s mult_s * sum_{y,x} wy_s(y) wx_s(x) * ln(1 + exp(w_s(y,x)))
    w_s = (1 - 2*E_s) * Corr_s,   Corr_s(y,x) = sum_c A[c,y,x] A[c,y+dr,x+dc],
    E_s = [labels equal across the shift]

(BCE-with-logits == softplus((1-2E)*x); max |Corr| ~ 66 so exp is safe in f32.)

Sharding: data-parallel, core k owns image k//2, rows [192*(k%2), +192).
Per core: pass0 = 128 rows full width; pass1 = 64 rows x 2 x-half groups
stacked on partitions. Per shift: bf16 product tile [128,19,W] (DVE mult, or
ScalarE Square for s=(0,0)) -> bf16 tree-add over c -> corr; sign w from
labels (is_equal + affine + mult); ScalarE Exp then Ln(bias=1, accum_out) give
the per-row loss sums. wx border deviations live within 2 cols of the edges;
those columns are exported exactly (GPSIMD copies) and corrected on host in
f64 together with the wy weighting and the cross-core reduction.
"""

import os
import sys

import numpy as np

for _p in ("/root/.axon_site", "/root/.axon_site/_ro/trn_rl_repo",
           "/root/.axon_site/_ro/pypackages"):
    if os.path.isdir(_p) and _p not in sys.path:
        sys.path.append(_p)

import ml_dtypes  # noqa: E402

N, C, H, W = 4, 19, 384, 384
K = 3
HP = WP = H - K + 1  # 382
N_CORES = 8
ROWS_PER_CORE = 192
SHIFTS = [(0, 0), (0, 1), (0, 2)] + [(dr, dc) for dr in (1, 2) for dc in (-2, -1, 0, 1, 2)]
NS = len(SHIFTS)  # 13
PASS_GEOM = [
    dict(row0=0, DW=W, TW=W + 8, groups=1),      # tiles [128, C, 392], data cols 2..385
    dict(row0=128, DW=194, TW=200, groups=2),    # 64 rows x 2 x-groups
]
GROUP_X0 = [0, 190]   # pass1 group g covers x in [190g, 190g+194)
COLS = 2 * NS        # one ln-accum column per (pass, shift)
BCOLS = 2 * NS * 4   # 4 exported border cols per (pass, shift)
LGROWS = 196


def _wx_profile(dc, x):
    w = np.zeros_like(x, dtype=np.float64)
    for ca in range(K):
        if 0 <= ca + dc < K:
            w += ((x - ca >= 0) & (x - ca < WP))
    return w


def _wy_profile(dr, y):
    w = np.zeros_like(y, dtype=np.float64)
    for ra in range(K):
        if 0 <= ra + dr < K:
            w += ((y - ra >= 0) & (y - ra < HP))
    return w


def _border_weights():
    """bw[p, (pass*NS+s)*4 + bi]: (wx_eff - wxc) at window cols {0,1,DW-2,DW-1}."""
    bw = np.zeros((128, 2 * NS * 4), np.float64)
    for pi, geom in enumerate(PASS_GEOM):
        DW = geom["DW"]
        for si, (dr, dc) in enumerate(SHIFTS):
            wxc = sum(1 for ca in range(K) if 0 <= ca + dc < K)
            for p in range(128):
                if pi == 0:
                    gx0, own_lo, own_hi = 0, 0, W
                else:
                    g = p // 64
                    gx0 = GROUP_X0[g]
                    own_lo, own_hi = (0, 192) if g == 0 else (192, W)
                for bi, j in enumerate((0, 1, DW - 2, DW - 1)):
                    x = gx0 + j
                    if own_lo <= x < own_hi and 0 <= x + dc < W and x < W:
                        wx = _wx_profile(dc, np.array([x]))[0]
                    else:
                        wx = 0.0
                    bw[p, (pi * NS + si) * 4 + bi] = wx - wxc
    return bw


_BW = None


def _host_inputs(logits, labels):
    in_maps = []
    for k in range(N_CORES):
        img, half = k // 2, k % 2
        g0 = half * ROWS_PER_CORE
        hi = min(H, g0 + LGROWS)
        lg = np.zeros((C, LGROWS, W), np.float32)
        lg[:, : hi - g0] = logits[img, :, g0:hi]
        lb = np.full((LGROWS, W), -1.0, np.float32)
        lb[: hi - g0] = labels[img, g0:hi].astype(np.float32)
        in_maps.append({
            "lg": lg.astype(ml_dtypes.bfloat16),
            "lb": lb.astype(ml_dtypes.bfloat16),
        })
    return in_maps


def _combine(accs_list, bcols_list):
    global _BW
    if _BW is None:
        _BW = _border_weights()
    total = 0.0
    for k in range(N_CORES):
        acc = accs_list[k].astype(np.float64)
        bc = bcols_list[k].astype(np.float64)
        g0 = (k % 2) * ROWS_PER_CORE
        for pi in range(2):
            p = np.arange(128)
            gy = g0 + p if pi == 0 else g0 + 128 + (p % 64)
            for si, (dr, dc) in enumerate(SHIFTS):
                mult = 1.0 if (dr, dc) == (0, 0) else 2.0
                wxc = float(sum(1 for ca in range(K) if 0 <= ca + dc < K))
                wy = _wy_profile(dr, gy)
                idx = pi * NS + si
                wb = _BW[:, idx * 4: idx * 4 + 4]
                full = acc[:, idx]
                border = (bc[:, idx * 4: idx * 4 + 4] * wb).sum(1)
                total += mult * np.sum(wy * (wxc * full + border))
    return total / (N * 81 * HP * WP)


_NC = None


def _build():
    global _NC
    if _NC is not None:
        return _NC
    from concourse import bacc, mybir
    import concourse.tile as tile

    f32 = mybir.dt.float32
    bf16 = mybir.dt.bfloat16
    Alu = mybir.AluOpType
    AF = mybir.ActivationFunctionType

    # All activations used here (Exp, Ln, Square, Copy) live together in the
    # "natural_log_exp_and_others" table set, but the table-load pass resolves
    # each function to the FIRST set containing it, which alternates sets and
    # reloads the ACT tables before nearly every activation (~80us of
    # ACT_TABLE_LOAD).  Filter the other sets' membership (indices untouched)
    # so everything resolves to the one shared set -> a single load.
    from concourse.hw_specs import get_activation_tables as _gat
    _keep = "natural_log_exp_and_others"
    _mine = {AF.Exp, AF.Ln, AF.Square, AF.Copy}

    def _gat_filtered(arch):
        t = _gat(arch)
        for name in t:
            if name != _keep:
                t[name] = t[name] - _mine
        return t

    bacc.get_activation_tables = _gat_filtered

    nc = bacc.Bacc("TRN2", target_bir_lowering=False, debug=False, num_devices=N_CORES)
    lg = nc.dram_tensor("lg", (C, LGROWS, W), bf16, kind="ExternalInput")
    lb = nc.dram_tensor("lb", (LGROWS, W), bf16, kind="ExternalInput")
    accs = nc.dram_tensor("accs", (128, COLS), f32, kind="ExternalOutput")
    bcols = nc.dram_tensor("bcols", (128, BCOLS), f32, kind="ExternalOutput")

    with tile.TileContext(nc) as tc:
        with tc.tile_pool(name="persist", bufs=1) as pool, \
             tc.tile_pool(name="work", bufs=2) as wpool:
            accs_t = pool.tile([128, COLS], f32, name="accs_t")
            bcols_t = pool.tile([128, BCOLS], f32, name="bcols_t")

            for pi, geom in enumerate(PASS_GEOM):
                row0, DW, TW, groups = geom["row0"], geom["DW"], geom["TW"], geom["groups"]
                T, Tod, L = {}, {}, {}
                for dr in range(K):
                    t = pool.tile([128, C, TW], bf16, tag=f"T{dr}_{pi}", name=f"T{dr}_{pi}")
                    to = pool.tile([128, C, TW], bf16, tag=f"O{dr}_{pi}", name=f"O{dr}_{pi}")
                    lt = pool.tile([128, TW], bf16, tag=f"L{dr}_{pi}", name=f"L{dr}_{pi}")
                    nc.gpsimd.memset(t[:, :, 0:2], 0)
                    nc.gpsimd.memset(t[:, :, 2 + DW:TW], 0)
                    nc.gpsimd.memset(to[:, :, 0:1], 0)
                    nc.gpsimd.memset(to[:, :, 1 + DW:TW], 0)
                    nc.gpsimd.memset(lt[:, 0:2], 0)
                    nc.gpsimd.memset(lt[:, 2 + DW:TW], 0)
                    if groups == 1:
                        # chunk over c so the load spreads across 4 DMA queues
                        # (a single-queue 1.9MB dma_start would gate the first
                        # compute op by ~27us)
                        for c0, c1 in ((0, 5), (5, 10), (10, 15), (15, C)):
                            src = lg[c0:c1, row0 + dr:row0 + dr + 128, :].rearrange(
                                "c y x -> y c x")
                            nc.sync.dma_start(t[:, c0:c1, 2:2 + DW], src)
                            nc.sync.dma_start(to[:, c0:c1, 1:1 + DW], src)
                        nc.sync.dma_start(lt[:, 2:2 + DW], lb[row0 + dr:row0 + dr + 128, :])
                    else:
                        for g in range(groups):
                            x0 = GROUP_X0[g]
                            src = lg[:, row0 + dr:row0 + dr + 64, x0:x0 + DW].rearrange(
                                "c y x -> y c x")
                            nc.sync.dma_start(t[64 * g:64 * g + 64, :, 2:2 + DW], src)
                            nc.sync.dma_start(to[64 * g:64 * g + 64, :, 1:1 + DW], src)
                            nc.sync.dma_start(
                                lt[64 * g:64 * g + 64, 2:2 + DW],
                                lb[row0 + dr:row0 + dr + 64, x0:x0 + DW])
                    T[dr], Tod[dr], L[dr] = t, to, lt

                for si, (dr, dc) in enumerate(SHIFTS):
                    idx = pi * NS + si
                    odd = (dc % 2) != 0
                    t1 = Tod[dr] if odd else T[dr]
                    o1 = (1 + dc) if odd else (2 + dc)

                    pb = wpool.tile([128, C, DW], bf16, tag="pb", name=f"pb_{pi}_{si}")
                    corr = wpool.tile([128, DW], bf16, tag="corr", name=f"corr_{pi}_{si}")
                    u = wpool.tile([128, DW], f32, tag="u", name=f"u_{pi}_{si}")
                    l1 = wpool.tile([128, DW], f32, tag="l1", name=f"l1_{pi}_{si}")

                    if si == 0:
                        # Corr_00 = sum_c A_c^2: products on ScalarE (Square)
                        nc.scalar.activation(
                            pb[:, :, :], T[0][:, :, 2:2 + DW], AF.Square)
                    else:
                        nc.vector.tensor_tensor(
                            pb[:, :, :], T[0][:, :, 2:2 + DW], t1[:, :, o1:o1 + DW],
                            Alu.mult)
                    nc.vector.tensor_tensor(
                        pb[:, 0:9, :], pb[:, 0:9, :], pb[:, 9:18, :], Alu.add)
                    nc.vector.tensor_tensor(
                        pb[:, 0:4, :], pb[:, 0:4, :], pb[:, 4:8, :], Alu.add)
                    nc.vector.tensor_tensor(
                        pb[:, 0:2, :], pb[:, 0:2, :], pb[:, 2:4, :], Alu.add)
                    nc.vector.tensor_tensor(
                        pb[:, 0:1, :], pb[:, 0:1, :], pb[:, 1:2, :], Alu.add)
                    nc.vector.tensor_tensor(
                        pb[:, 0:1, :], pb[:, 0:1, :], pb[:, 8:9, :], Alu.add)
                    nc.vector.tensor_tensor(
                        corr[:, 0:DW], pb[:, 0:1, :], pb[:, 18:19, :], Alu.add)

                    if si == 0:
                        # E == 1 everywhere: w = -corr, fold into Exp's scale
                        nc.scalar.activation(u[:, 0:DW], corr[:, 0:DW], AF.Exp,
                                             scale=-1.0)
                    else:
                        e = wpool.tile([128, DW], bf16, tag="e", name=f"e_{pi}_{si}")
                        sh = wpool.tile([128, DW], bf16, tag="sh", name=f"sh_{pi}_{si}")
                        wt = wpool.tile([128, DW], bf16, tag="wt", name=f"wt_{pi}_{si}")
                        nc.vector.tensor_tensor(
                            e[:, 0:DW], L[0][:, 2:2 + DW], L[dr][:, 2 + dc:2 + dc + DW],
                            Alu.is_equal)
                        nc.vector.tensor_scalar(
                            sh[:, 0:DW], e[:, 0:DW], -2.0, 1.0, Alu.mult, Alu.add)
                        nc.vector.tensor_tensor(
                            wt[:, 0:DW], sh[:, 0:DW], corr[:, 0:DW], Alu.mult)
                        nc.scalar.activation(u[:, 0:DW], wt[:, 0:DW], AF.Exp)
                    nc.scalar.activation(
                        l1[:, 0:DW], u[:, 0:DW], AF.Ln, bias=1.0,
                        accum_out=accs_t[:, idx:idx + 1])
                    nc.scalar.copy(bcols_t[:, idx * 4:idx * 4 + 2], l1[:, 0:2])
                    nc.scalar.copy(bcols_t[:, idx * 4 + 2:idx * 4 + 4],
                                   l1[:, DW - 2:DW])

            nc.sync.dma_start(accs[:, :], accs_t[:])
            nc.sync.dma_start(bcols[:, :], bcols_t[:])

    nc.finalize()
    _NC = nc
    return nc


def kernel(logits, labels):
    nc = _build()
    in_maps = _host_inputs(np.asarray(logits, np.float32), np.asarray(labels))
    from concourse.bass_utils import run_bass_kernel_spmd
    res = run_bass_kernel_spmd(nc, in_maps, core_ids=list(range(N_CORES)))
    accs_list = [res.results[k]["accs"] for k in range(N_CORES)]
    bcols_list = [res.results[k]["bcols"] for k in range(N_CORES)]
    return np.array(_combine(accs_list, bcols_list), np.float32)


# revision 17
# speedup vs baseline: 1.2021x; 1.0039x over previous
"""AffinityLoss on 8 TRN2 NeuronCores (Bass/Tile).

Math: 3x3-unfold affinity loss = mean BCE-with-logits between per-pixel 9x9
channel Gram matrices and label-equality maps. Reformulated over 13 canonical
relative shifts s=(dr,dc) with integer edge-weight profiles wy/wx:

    loss_sum = sum_s mult_s * sum_{y,x} wy_s(y) wx_s(x) * ln(1 + exp(w_s(y,x)))
    w_s = (1 - 2*E_s) * Corr_s,   Corr_s(y,x) = sum_c A[c,y,x] A[c,y+dr,x+dc],
    E_s = [labels equal across the shift]

(BCE-with-logits == softplus((1-2E)*x); max |Corr| ~ 66 so exp is safe in f32.)

Sharding: data-parallel, core k owns image k//2, rows [192*(k%2), +192).
Per core: pass0 = 128 rows full width; pass1 = 64 rows x 2 x-half groups
stacked on partitions. Per shift: bf16 product tile [128,19,W] (DVE mult, or
ScalarE Square for s=(0,0)) -> bf16 tree-add over c -> corr; sign w from
labels (is_equal + affine + mult); ScalarE Exp then Ln(bias=1, accum_out) give
the per-row loss sums. wx border deviations live within 2 cols of the edges;
those columns are exported exactly (GPSIMD copies) and corrected on host in
f64 together with the wy weighting and the cross-core reduction.
"""

import os
import sys

import numpy as np

for _p in ("/root/.axon_site", "/root/.axon_site/_ro/trn_rl_repo",
           "/root/.axon_site/_ro/pypackages"):
    if os.path.isdir(_p) and _p not in sys.path:
        sys.path.append(_p)

import ml_dtypes  # noqa: E402

N, C, H, W = 4, 19, 384, 384
K = 3
HP = WP = H - K + 1  # 382
N_CORES = 8
ROWS_PER_CORE = 192
SHIFTS = [(0, 0), (0, 1), (0, 2)] + [(dr, dc) for dr in (1, 2) for dc in (-2, -1, 0, 1, 2)]
NS = len(SHIFTS)  # 13
PASS_GEOM = [
    dict(row0=0, DW=W, TW=W + 8, groups=1),      # tiles [128, C, 392], data cols 2..385
    dict(row0=128, DW=194, TW=200, groups=2),    # 64 rows x 2 x-groups
]
GROUP_X0 = [0, 190]   # pass1 group g covers x in [190g, 190g+194)
COLS = 2 * NS        # one ln-accum column per (pass, shift)
BCOLS = 2 * NS * 4   # 4 exported border cols per (pass, shift)
LGROWS = 196


def _wx_profile(dc, x):
    w = np.zeros_like(x, dtype=np.float64)
    for ca in range(K):
        if 0 <= ca + dc < K:
            w += ((x - ca >= 0) & (x - ca < WP))
    return w


def _wy_profile(dr, y):
    w = np.zeros_like(y, dtype=np.float64)
    for ra in range(K):
        if 0 <= ra + dr < K:
            w += ((y - ra >= 0) & (y - ra < HP))
    return w


def _border_weights():
    """bw[p, (pass*NS+s)*4 + bi]: (wx_eff - wxc) at window cols {0,1,DW-2,DW-1}."""
    bw = np.zeros((128, 2 * NS * 4), np.float64)
    for pi, geom in enumerate(PASS_GEOM):
        DW = geom["DW"]
        for si, (dr, dc) in enumerate(SHIFTS):
            wxc = sum(1 for ca in range(K) if 0 <= ca + dc < K)
            for p in range(128):
                if pi == 0:
                    gx0, own_lo, own_hi = 0, 0, W
                else:
                    g = p // 64
                    gx0 = GROUP_X0[g]
                    own_lo, own_hi = (0, 192) if g == 0 else (192, W)
                for bi, j in enumerate((0, 1, DW - 2, DW - 1)):
                    x = gx0 + j
                    if own_lo <= x < own_hi and 0 <= x + dc < W and x < W:
                        wx = _wx_profile(dc, np.array([x]))[0]
                    else:
                        wx = 0.0
                    bw[p, (pi * NS + si) * 4 + bi] = wx - wxc
    return bw


_BW = None


def _host_inputs(logits, labels):
    in_maps = []
    for k in range(N_CORES):
        img, half = k // 2, k % 2
        g0 = half * ROWS_PER_CORE
        hi = min(H, g0 + LGROWS)
        lg = np.zeros((C, LGROWS, W), np.float32)
        lg[:, : hi - g0] = logits[img, :, g0:hi]
        lb = np.full((LGROWS, W), -1.0, np.float32)
        lb[: hi - g0] = labels[img, g0:hi].astype(np.float32)
        in_maps.append({
            "lg": lg.astype(ml_dtypes.bfloat16),
            "lb": lb.astype(ml_dtypes.bfloat16),
        })
    return in_maps


def _combine(accs_list, bcols_list):
    global _BW
    if _BW is None:
        _BW = _border_weights()
    total = 0.0
    for k in range(N_CORES):
        acc = accs_list[k].astype(np.float64)
        bc = bcols_list[k].astype(np.float64)
        g0 = (k % 2) * ROWS_PER_CORE
        for pi in range(2):
            p = np.arange(128)
            gy = g0 + p if pi == 0 else g0 + 128 + (p % 64)
            for si, (dr, dc) in enumerate(SHIFTS):
                mult = 1.0 if (dr, dc) == (0, 0) else 2.0
                wxc = float(sum(1 for ca in range(K) if 0 <= ca + dc < K))
                wy = _wy_profile(dr, gy)
                idx = pi * NS + si
                wb = _BW[:, idx * 4: idx * 4 + 4]
                full = acc[:, idx]
                border = (bc[:, idx * 4: idx * 4 + 4] * wb).sum(1)
                total += mult * np.sum(wy * (wxc * full + border))
    return total / (N * 81 * HP * WP)


_NC = None


def _build():
    global _NC
    if _NC is not None:
        return _NC
    from concourse import bacc, mybir
    import concourse.tile as tile

    f32 = mybir.dt.float32
    bf16 = mybir.dt.bfloat16
    Alu = mybir.AluOpType
    AF = mybir.ActivationFunctionType

    # All activations used here (Exp, Ln, Square, Copy) live together in the
    # "natural_log_exp_and_others" table set, but the table-load pass resolves
    # each function to the FIRST set containing it, which alternates sets and
    # reloads the ACT tables before nearly every activation (~80us of
    # ACT_TABLE_LOAD).  Filter the other sets' membership (indices untouched)
    # so everything resolves to the one shared set -> a single load.
    from concourse.hw_specs import get_activation_tables as _gat
    _keep = "natural_log_exp_and_others"
    _mine = {AF.Exp, AF.Ln, AF.Square, AF.Copy}

    def _gat_filtered(arch):
        t = _gat(arch)
        for name in t:
            if name != _keep:
                t[name] = t[name] - _mine
        return t

    bacc.get_activation_tables = _gat_filtered

    nc = bacc.Bacc("TRN2", target_bir_lowering=False, debug=False, num_devices=N_CORES)
    lg = nc.dram_tensor("lg", (C, LGROWS, W), bf16, kind="ExternalInput")
    lb = nc.dram_tensor("lb", (LGROWS, W), bf16, kind="ExternalInput")
    accs = nc.dram_tensor("accs", (128, COLS), f32, kind="ExternalOutput")
    bcols = nc.dram_tensor("bcols", (128, BCOLS), f32, kind="ExternalOutput")

    with tile.TileContext(nc) as tc:
        with tc.tile_pool(name="persist", bufs=1) as pool, \
             tc.tile_pool(name="work", bufs=2) as wpool:
            accs_t = pool.tile([128, COLS], f32, name="accs_t")
            bcols_t = pool.tile([128, BCOLS], f32, name="bcols_t")

            for pi, geom in enumerate(PASS_GEOM):
                row0, DW, TW, groups = geom["row0"], geom["DW"], geom["TW"], geom["groups"]
                T, Tod, L = {}, {}, {}
                for dr in range(K):
                    t = pool.tile([128, C, TW], bf16, tag=f"T{dr}_{pi}", name=f"T{dr}_{pi}")
                    to = pool.tile([128, C, TW], bf16, tag=f"O{dr}_{pi}", name=f"O{dr}_{pi}")
                    lt = pool.tile([128, TW], bf16, tag=f"L{dr}_{pi}", name=f"L{dr}_{pi}")
                    nc.gpsimd.memset(t[:, :, 0:2], 0)
                    nc.gpsimd.memset(t[:, :, 2 + DW:TW], 0)
                    nc.gpsimd.memset(to[:, :, 0:1], 0)
                    nc.gpsimd.memset(to[:, :, 1 + DW:TW], 0)
                    nc.gpsimd.memset(lt[:, 0:2], 0)
                    nc.gpsimd.memset(lt[:, 2 + DW:TW], 0)
                    if groups == 1:
                        # per-queue DMA is ~22GB/s, so the first tiles are
                        # chunked across many queues to cut arrival latency
                        # (compute starts as soon as T0 lands)
                        nchunk = 16 if dr == 0 else 8
                        bnds = [round(C * i / nchunk) for i in range(nchunk + 1)]
                        for c0, c1 in zip(bnds[:-1], bnds[1:]):
                            if c0 == c1:
                                continue
                            src = lg[c0:c1, row0 + dr:row0 + dr + 128, :].rearrange(
                                "c y x -> y c x")
                            nc.sync.dma_start(t[:, c0:c1, 2:2 + DW], src)
                        nc.sync.dma_start(lt[:, 2:2 + DW], lb[row0 + dr:row0 + dr + 128, :])
                        for c0, c1 in zip(bnds[:-1], bnds[1:]):
                            if c0 == c1:
                                continue
                            src = lg[c0:c1, row0 + dr:row0 + dr + 128, :].rearrange(
                                "c y x -> y c x")
                            nc.sync.dma_start(to[:, c0:c1, 1:1 + DW], src)
                    else:
                        for g in range(groups):
                            x0 = GROUP_X0[g]
                            for c0, c1 in ((0, 10), (10, C)):
                                src = lg[c0:c1, row0 + dr:row0 + dr + 64, x0:x0 + DW
                                         ].rearrange("c y x -> y c x")
                                nc.sync.dma_start(
                                    t[64 * g:64 * g + 64, c0:c1, 2:2 + DW], src)
                                nc.sync.dma_start(
                                    to[64 * g:64 * g + 64, c0:c1, 1:1 + DW], src)
                            nc.sync.dma_start(
                                lt[64 * g:64 * g + 64, 2:2 + DW],
                                lb[row0 + dr:row0 + dr + 64, x0:x0 + DW])
                    T[dr], Tod[dr], L[dr] = t, to, lt

                # iterate shifts in data-arrival order: T0-only shifts first,
                # then Tod0, T1, Tod1, T2, Tod2
                order = [2, 0, 1, 5, 7, 3, 6, 4, 10, 12, 8, 11, 9]
                for si in order:
                    dr, dc = SHIFTS[si]
                    idx = pi * NS + si
                    odd = (dc % 2) != 0
                    t1 = Tod[dr] if odd else T[dr]
                    o1 = (1 + dc) if odd else (2 + dc)

                    pb = wpool.tile([128, C, DW], bf16, tag="pb", name=f"pb_{pi}_{si}")
                    corr = wpool.tile([128, DW], bf16, tag="corr", name=f"corr_{pi}_{si}")
                    u = wpool.tile([128, DW], f32, tag="u", name=f"u_{pi}_{si}")
                    l1 = wpool.tile([128, DW], f32, tag="l1", name=f"l1_{pi}_{si}")

                    if si == 0:
                        # Corr_00 = sum_c A_c^2: products on ScalarE (Square)
                        nc.scalar.activation(
                            pb[:, :, :], T[0][:, :, 2:2 + DW], AF.Square)
                    else:
                        nc.vector.tensor_tensor(
                            pb[:, :, :], T[0][:, :, 2:2 + DW], t1[:, :, o1:o1 + DW],
                            Alu.mult)
                    nc.vector.tensor_tensor(
                        pb[:, 0:9, :], pb[:, 0:9, :], pb[:, 9:18, :], Alu.add)
                    nc.vector.tensor_tensor(
                        pb[:, 0:4, :], pb[:, 0:4, :], pb[:, 4:8, :], Alu.add)
                    nc.vector.tensor_tensor(
                        pb[:, 0:2, :], pb[:, 0:2, :], pb[:, 2:4, :], Alu.add)
                    nc.vector.tensor_tensor(
                        pb[:, 0:1, :], pb[:, 0:1, :], pb[:, 1:2, :], Alu.add)
                    nc.vector.tensor_tensor(
                        pb[:, 0:1, :], pb[:, 0:1, :], pb[:, 8:9, :], Alu.add)
                    nc.vector.tensor_tensor(
                        corr[:, 0:DW], pb[:, 0:1, :], pb[:, 18:19, :], Alu.add)

                    if si == 0:
                        # E == 1 everywhere: w = -corr, fold into Exp's scale
                        nc.scalar.activation(u[:, 0:DW], corr[:, 0:DW], AF.Exp,
                                             scale=-1.0)
                    else:
                        e = wpool.tile([128, DW], bf16, tag="e", name=f"e_{pi}_{si}")
                        sh = wpool.tile([128, DW], bf16, tag="sh", name=f"sh_{pi}_{si}")
                        wt = wpool.tile([128, DW], bf16, tag="wt", name=f"wt_{pi}_{si}")
                        nc.vector.tensor_tensor(
                            e[:, 0:DW], L[0][:, 2:2 + DW], L[dr][:, 2 + dc:2 + dc + DW],
                            Alu.is_equal)
                        nc.vector.tensor_scalar(
                            sh[:, 0:DW], e[:, 0:DW], -2.0, 1.0, Alu.mult, Alu.add)
                        nc.vector.tensor_tensor(
                            wt[:, 0:DW], sh[:, 0:DW], corr[:, 0:DW], Alu.mult)
                        nc.scalar.activation(u[:, 0:DW], wt[:, 0:DW], AF.Exp)
                    nc.scalar.activation(
                        l1[:, 0:DW], u[:, 0:DW], AF.Ln, bias=1.0,
                        accum_out=accs_t[:, idx:idx + 1])
                    nc.scalar.copy(bcols_t[:, idx * 4:idx * 4 + 2], l1[:, 0:2])
                    nc.scalar.copy(bcols_t[:, idx * 4 + 2:idx * 4 + 4],
                                   l1[:, DW - 2:DW])

                # flush this pass's halves so the kernel tail is short
                nc.sync.dma_start(accs[:, pi * NS:(pi + 1) * NS],
                                  accs_t[:, pi * NS:(pi + 1) * NS])
                nc.sync.dma_start(bcols[:, pi * NS * 4:(pi + 1) * NS * 4],
                                  bcols_t[:, pi * NS * 4:(pi + 1) * NS * 4])

    nc.finalize()
    _NC = nc
    return nc


def kernel(logits, labels):
    nc = _build()
    in_maps = _host_inputs(np.asarray(logits, np.float32), np.asarray(labels))
    from concourse.bass_utils import run_bass_kernel_spmd
    res = run_bass_kernel_spmd(nc, in_maps, core_ids=list(range(N_CORES)))
    accs_list = [res.results[k]["accs"] for k in range(N_CORES)]
    bcols_list = [res.results[k]["bcols"] for k in range(N_CORES)]
    return np.array(_combine(accs_list, bcols_list), np.float32)


# revision 19
# speedup vs baseline: 1.4485x; 1.2050x over previous
"""AffinityLoss on 8 TRN2 NeuronCores (Bass/Tile).

Math: 3x3-unfold affinity loss = mean BCE-with-logits between per-pixel 9x9
channel Gram matrices and label-equality maps. Reformulated over 13 canonical
relative shifts s=(dr,dc) with integer edge-weight profiles wy/wx:

    loss_sum = sum_s mult_s * sum_{y,x} wy_s(y) wx_s(x) * ln(1 + exp(w_s(y,x)))
    w_s = (1 - 2*E_s) * Corr_s,   Corr_s(y,x) = sum_c A[c,y,x] A[c,y+dr,x+dc],
    E_s = [labels equal across the shift]

(BCE-with-logits == softplus((1-2E)*x); max |Corr| ~ 66 so exp is safe in f32.)

Sharding: data-parallel, core k owns image k//2, rows [192*(k%2), +192).
Per core: pass0 = 128 rows full width; pass1 = 64 rows x 2 x-half groups
stacked on partitions. Per shift: bf16 product tile [128,19,W] (DVE mult, or
ScalarE Square for s=(0,0)) -> bf16 tree-add over c -> corr; sign w from
labels (is_equal + affine + mult); ScalarE Exp then Ln(bias=1, accum_out) give
the per-row loss sums. wx border deviations live within 2 cols of the edges;
those columns are exported exactly (GPSIMD copies) and corrected on host in
f64 together with the wy weighting and the cross-core reduction.
"""

import os
import sys

import numpy as np

for _p in ("/root/.axon_site", "/root/.axon_site/_ro/trn_rl_repo",
           "/root/.axon_site/_ro/pypackages"):
    if os.path.isdir(_p) and _p not in sys.path:
        sys.path.append(_p)

import ml_dtypes  # noqa: E402

N, C, H, W = 4, 19, 384, 384
K = 3
HP = WP = H - K + 1  # 382
N_CORES = 8
ROWS_PER_CORE = 192
SHIFTS = [(0, 0), (0, 1), (0, 2)] + [(dr, dc) for dr in (1, 2) for dc in (-2, -1, 0, 1, 2)]
NS = len(SHIFTS)  # 13
PASS_GEOM = [
    dict(row0=0, DW=W, TW=W + 8, groups=1),      # tiles [128, C, 392], data cols 2..385
    dict(row0=128, DW=194, TW=200, groups=2),    # 64 rows x 2 x-groups
]
GROUP_X0 = [0, 190]   # pass1 group g covers x in [190g, 190g+194)
COLS = 2 * NS        # one ln-accum column per (pass, shift)
BCOLS = 2 * NS * 4   # 4 exported border cols per (pass, shift)
LGROWS = 196


def _wx_profile(dc, x):
    w = np.zeros_like(x, dtype=np.float64)
    for ca in range(K):
        if 0 <= ca + dc < K:
            w += ((x - ca >= 0) & (x - ca < WP))
    return w


def _wy_profile(dr, y):
    w = np.zeros_like(y, dtype=np.float64)
    for ra in range(K):
        if 0 <= ra + dr < K:
            w += ((y - ra >= 0) & (y - ra < HP))
    return w


def _border_weights():
    """bw[p, (pass*NS+s)*4 + bi]: (wx_eff - wxc) at window cols {0,1,DW-2,DW-1}."""
    bw = np.zeros((128, 2 * NS * 4), np.float64)
    for pi, geom in enumerate(PASS_GEOM):
        DW = geom["DW"]
        for si, (dr, dc) in enumerate(SHIFTS):
            wxc = sum(1 for ca in range(K) if 0 <= ca + dc < K)
            for p in range(128):
                if pi == 0:
                    gx0, own_lo, own_hi = 0, 0, W
                else:
                    g = p // 64
                    gx0 = GROUP_X0[g]
                    own_lo, own_hi = (0, 192) if g == 0 else (192, W)
                for bi, j in enumerate((0, 1, DW - 2, DW - 1)):
                    x = gx0 + j
                    if own_lo <= x < own_hi and 0 <= x + dc < W and x < W:
                        wx = _wx_profile(dc, np.array([x]))[0]
                    else:
                        wx = 0.0
                    bw[p, (pi * NS + si) * 4 + bi] = wx - wxc
    return bw


_BW = None


def _host_inputs(logits, labels):
    in_maps = []
    for k in range(N_CORES):
        img, half = k // 2, k % 2
        g0 = half * ROWS_PER_CORE
        hi = min(H, g0 + LGROWS)
        lg = np.zeros((C, LGROWS, W), np.float32)
        lg[:, : hi - g0] = logits[img, :, g0:hi]
        lb = np.full((LGROWS, W), -1.0, np.float32)
        lb[: hi - g0] = labels[img, g0:hi].astype(np.float32)
        in_maps.append({
            "lg": lg.astype(ml_dtypes.bfloat16),
            "lb": lb.astype(ml_dtypes.bfloat16),
        })
    return in_maps


def _combine(accs_list, bcols_list):
    global _BW
    if _BW is None:
        _BW = _border_weights()
    total = 0.0
    for k in range(N_CORES):
        acc = accs_list[k].astype(np.float64)
        bc = bcols_list[k].astype(np.float64)
        g0 = (k % 2) * ROWS_PER_CORE
        for pi in range(2):
            p = np.arange(128)
            gy = g0 + p if pi == 0 else g0 + 128 + (p % 64)
            for si, (dr, dc) in enumerate(SHIFTS):
                mult = 1.0 if (dr, dc) == (0, 0) else 2.0
                wxc = float(sum(1 for ca in range(K) if 0 <= ca + dc < K))
                wy = _wy_profile(dr, gy)
                idx = pi * NS + si
                wb = _BW[:, idx * 4: idx * 4 + 4]
                full = acc[:, idx]
                border = (bc[:, idx * 4: idx * 4 + 4] * wb).sum(1)
                total += mult * np.sum(wy * (wxc * full + border))
    return total / (N * 81 * HP * WP)


_NC = None


def _build():
    global _NC
    if _NC is not None:
        return _NC
    from concourse import bacc, mybir
    import concourse.tile as tile

    f32 = mybir.dt.float32
    bf16 = mybir.dt.bfloat16
    Alu = mybir.AluOpType
    AF = mybir.ActivationFunctionType

    # All activations used here (Exp, Ln, Square, Copy) live together in the
    # "natural_log_exp_and_others" table set, but the table-load pass resolves
    # each function to the FIRST set containing it, which alternates sets and
    # reloads the ACT tables before nearly every activation (~80us of
    # ACT_TABLE_LOAD).  Filter the other sets' membership (indices untouched)
    # so everything resolves to the one shared set -> a single load.
    from concourse.hw_specs import get_activation_tables as _gat
    _keep = "natural_log_exp_and_others"
    _mine = {AF.Exp, AF.Ln, AF.Square, AF.Copy}

    def _gat_filtered(arch):
        t = _gat(arch)
        for name in t:
            if name != _keep:
                t[name] = t[name] - _mine
        return t

    bacc.get_activation_tables = _gat_filtered

    nc = bacc.Bacc("TRN2", target_bir_lowering=False, debug=False, num_devices=N_CORES)
    lg = nc.dram_tensor("lg", (C, LGROWS, W), bf16, kind="ExternalInput")
    lb = nc.dram_tensor("lb", (LGROWS, W), bf16, kind="ExternalInput")
    accs = nc.dram_tensor("accs", (128, COLS), f32, kind="ExternalOutput")
    bcols = nc.dram_tensor("bcols", (128, BCOLS), f32, kind="ExternalOutput")

    with tile.TileContext(nc) as tc:
        with tc.tile_pool(name="persist", bufs=1) as pool, \
             tc.tile_pool(name="work", bufs=2) as wpool:
            accs_t = pool.tile([128, COLS], f32, name="accs_t")
            bcols_t = pool.tile([128, BCOLS], f32, name="bcols_t")

            for pi, geom in enumerate(PASS_GEOM):
                row0, DW, TW, groups = geom["row0"], geom["DW"], geom["TW"], geom["groups"]
                T, Tod, L = {}, {}, {}
                for dr in range(K):
                    t = pool.tile([128, C, TW], bf16, tag=f"T{dr}_{pi}", name=f"T{dr}_{pi}")
                    to = pool.tile([128, C, TW], bf16, tag=f"O{dr}_{pi}", name=f"O{dr}_{pi}")
                    lt = pool.tile([128, TW], bf16, tag=f"L{dr}_{pi}", name=f"L{dr}_{pi}")
                    nc.gpsimd.memset(t[:, :, 0:2], 0)
                    nc.gpsimd.memset(t[:, :, 2 + DW:TW], 0)
                    nc.gpsimd.memset(to[:, :, 0:1], 0)
                    nc.gpsimd.memset(to[:, :, 1 + DW:TW], 0)
                    nc.gpsimd.memset(lt[:, 0:2], 0)
                    nc.gpsimd.memset(lt[:, 2 + DW:TW], 0)
                    if groups == 1:
                        # per-queue DMA is ~22GB/s, so the first tiles are
                        # chunked across many queues to cut arrival latency
                        # (compute starts as soon as T0 lands)
                        nchunk = 16 if dr == 0 else 8
                        bnds = [round(C * i / nchunk) for i in range(nchunk + 1)]
                        for c0, c1 in zip(bnds[:-1], bnds[1:]):
                            if c0 == c1:
                                continue
                            src = lg[c0:c1, row0 + dr:row0 + dr + 128, :].rearrange(
                                "c y x -> y c x")
                            nc.sync.dma_start(t[:, c0:c1, 2:2 + DW], src)
                        nc.sync.dma_start(lt[:, 2:2 + DW], lb[row0 + dr:row0 + dr + 128, :])
                        for c0, c1 in zip(bnds[:-1], bnds[1:]):
                            if c0 == c1:
                                continue
                            src = lg[c0:c1, row0 + dr:row0 + dr + 128, :].rearrange(
                                "c y x -> y c x")
                            nc.sync.dma_start(to[:, c0:c1, 1:1 + DW], src)
                    else:
                        for g in range(groups):
                            x0 = GROUP_X0[g]
                            for c0, c1 in ((0, 10), (10, C)):
                                src = lg[c0:c1, row0 + dr:row0 + dr + 64, x0:x0 + DW
                                         ].rearrange("c y x -> y c x")
                                nc.sync.dma_start(
                                    t[64 * g:64 * g + 64, c0:c1, 2:2 + DW], src)
                                nc.sync.dma_start(
                                    to[64 * g:64 * g + 64, c0:c1, 1:1 + DW], src)
                            nc.sync.dma_start(
                                lt[64 * g:64 * g + 64, 2:2 + DW],
                                lb[row0 + dr:row0 + dr + 64, x0:x0 + DW])
                    T[dr], Tod[dr], L[dr] = t, to, lt

                # iterate shifts in data-arrival order: T0-only shifts first,
                # then Tod0, T1, Tod1, T2, Tod2
                order = [2, 0, 1, 5, 7, 3, 6, 4, 10, 12, 8, 11, 9]
                for si in order:
                    dr, dc = SHIFTS[si]
                    idx = pi * NS + si
                    odd = (dc % 2) != 0
                    t1 = Tod[dr] if odd else T[dr]
                    o1 = (1 + dc) if odd else (2 + dc)

                    pb = wpool.tile([128, C, DW], bf16, tag="pb", name=f"pb_{pi}_{si}")
                    corr = wpool.tile([128, DW], bf16, tag="corr", name=f"corr_{pi}_{si}")
                    u = wpool.tile([128, DW], f32, tag="u", name=f"u_{pi}_{si}")
                    l1 = wpool.tile([128, DW], f32, tag="l1", name=f"l1_{pi}_{si}")

                    # first shifts of pass0: chunk products over c so compute
                    # starts while the T0 chunks are still arriving
                    csplits = ((0, 5), (5, 10), (10, 14), (14, C)) \
                        if (pi == 0 and si in (0, 2)) else ((0, C),)
                    for c0, c1 in csplits:
                        if si == 0:
                            # Corr_00 = sum_c A_c^2: products on ScalarE (Square)
                            nc.scalar.activation(
                                pb[:, c0:c1, :], T[0][:, c0:c1, 2:2 + DW], AF.Square)
                        else:
                            nc.vector.tensor_tensor(
                                pb[:, c0:c1, :], T[0][:, c0:c1, 2:2 + DW],
                                t1[:, c0:c1, o1:o1 + DW], Alu.mult)
                    nc.vector.tensor_tensor(
                        pb[:, 0:9, :], pb[:, 0:9, :], pb[:, 9:18, :], Alu.add)
                    nc.vector.tensor_tensor(
                        pb[:, 0:4, :], pb[:, 0:4, :], pb[:, 4:8, :], Alu.add)
                    nc.vector.tensor_tensor(
                        pb[:, 0:2, :], pb[:, 0:2, :], pb[:, 2:4, :], Alu.add)
                    nc.vector.tensor_tensor(
                        pb[:, 0:1, :], pb[:, 0:1, :], pb[:, 1:2, :], Alu.add)
                    nc.vector.tensor_tensor(
                        pb[:, 0:1, :], pb[:, 0:1, :], pb[:, 8:9, :], Alu.add)
                    nc.vector.tensor_tensor(
                        corr[:, 0:DW], pb[:, 0:1, :], pb[:, 18:19, :], Alu.add)

                    if si == 0:
                        # E == 1 everywhere: w = -corr, fold into Exp's scale
                        nc.scalar.activation(u[:, 0:DW], corr[:, 0:DW], AF.Exp,
                                             scale=-1.0)
                    else:
                        e = wpool.tile([128, DW], bf16, tag="e", name=f"e_{pi}_{si}")
                        sh = wpool.tile([128, DW], bf16, tag="sh", name=f"sh_{pi}_{si}")
                        wt = wpool.tile([128, DW], bf16, tag="wt", name=f"wt_{pi}_{si}")
                        nc.vector.tensor_tensor(
                            e[:, 0:DW], L[0][:, 2:2 + DW], L[dr][:, 2 + dc:2 + dc + DW],
                            Alu.is_equal)
                        # sh = 1 - 2e via ScalarE's free affine (keeps DVE lean)
                        nc.scalar.activation(sh[:, 0:DW], e[:, 0:DW], AF.Identity,
                                             bias=1.0, scale=-2.0)
                        nc.vector.tensor_tensor(
                            wt[:, 0:DW], sh[:, 0:DW], corr[:, 0:DW], Alu.mult)
                        nc.scalar.activation(u[:, 0:DW], wt[:, 0:DW], AF.Exp)
                    nc.scalar.activation(
                        l1[:, 0:DW], u[:, 0:DW], AF.Ln, bias=1.0,
                        accum_out=accs_t[:, idx:idx + 1])
                    nc.scalar.copy(bcols_t[:, idx * 4:idx * 4 + 2], l1[:, 0:2])
                    nc.scalar.copy(bcols_t[:, idx * 4 + 2:idx * 4 + 4],
                                   l1[:, DW - 2:DW])

                # flush this pass's halves so the kernel tail is short
                nc.sync.dma_start(accs[:, pi * NS:(pi + 1) * NS],
                                  accs_t[:, pi * NS:(pi + 1) * NS])
                nc.sync.dma_start(bcols[:, pi * NS * 4:(pi + 1) * NS * 4],
                                  bcols_t[:, pi * NS * 4:(pi + 1) * NS * 4])

    nc.finalize()
    _NC = nc
    return nc


def kernel(logits, labels):
    nc = _build()
    in_maps = _host_inputs(np.asarray(logits, np.float32), np.asarray(labels))
    from concourse.bass_utils import run_bass_kernel_spmd
    res = run_bass_kernel_spmd(nc, in_maps, core_ids=list(range(N_CORES)))
    accs_list = [res.results[k]["accs"] for k in range(N_CORES)]
    bcols_list = [res.results[k]["bcols"] for k in range(N_CORES)]
    return np.array(_combine(accs_list, bcols_list), np.float32)
